# revision 13
# baseline (speedup 1.0000x reference)
"""EquivSetConv (hypergraph message passing) Trainium2 Bass kernel.

Math (reference):
  Xd = segment_sum(dif_vals * X[dif_cols], dif_rows, N)
  Xe = segment_sum((Xd@W1+b1)[vertex], edges, E)
  Xv = segment_sum(concat(Xd[vertex], Xe[edges]) @ W2 + b2, vertex, N)
  out = ((1-a)*Xv + a*Xd) @ W + b

Algebraic reassociation (exact up to fp reassociation), with
U1=(1-a)W2top@W, U2=(1-a)W2bot@W, U3=aW, V1=W1@U2, c1=b1@U2,
c2=(1-a)(b2@W):
  A[e]  = segment_sum(Xd[vertex], edges, E)
  Z     = A @ V1 + cnt_e x c1          (= Xe @ U2)
  B'[v] = segment_sum(Z[edges], vertex, N)
  out   = cnt_v o (Xd@U1) + Xd@U3 + B' + cnt_v x c2 + b

Distribution: nodes sharded 8 ways by row range; incidence lists bucketed by
destination core; the only collective is an AllReduce of the per-core partial
A^T [EG*64,128] bf16. Segment sums run as one-hot matmul accumulation in PSUM
over 128-destination groups; gathers use dma_gather (int16 idx, <=1024/call).
Step 2 uses swapped matmul operands (rows stationary, one-hot moving) so A is
produced transposed and step 3 needs no transposes. Step 5 is fused into
step 4's PSUM accumulation groups.
"""
import sys
import numpy as np

sys.path.insert(0, "/opt/trn_rl_repo")

D = 64
NC = 8
CHUNK = 1024        # dma_gather per-call token cap
MB = 16             # one-hot tiles built per DVE op
ALPHA = 0.5
BUCKET = 32768      # int16 gather index range
TRACE = False
LAST_EXEC_NS = None
LAST_RESULTS = None


def _wrap16(a):
    a = np.asarray(a, np.int16)
    return np.tile(a.reshape(-1, 16).T, (8, 1))  # [128, T/16]


def _wrap128(a):
    return np.ascontiguousarray(np.asarray(a).reshape(-1, 128).T)  # [128, T/128]


def _prep(inputs, n_edges):
    X = np.ascontiguousarray(np.asarray(inputs["X"], np.float32))
    N = X.shape[0]
    assert N % NC == 0
    S = N // NC
    G1 = -(-S // 128)          # node groups per core
    SP = G1 * 128
    EG = -(-n_edges // 128)    # edge groups
    EP = EG * 128
    NB = -(-N // BUCKET)

    dr = np.asarray(inputs["dif_rows"], np.int64)
    dc = np.asarray(inputs["dif_cols"], np.int64)
    dv = np.asarray(inputs["dif_vals"], np.float32)
    vx = np.asarray(inputs["vertex"], np.int64)
    eg = np.asarray(inputs["edges"], np.int64)
    assert eg.max() < n_edges and vx.max() < N and dr.max() < N and dc.max() < N

    # --- per-cell tile plans (max fill over cores; uniform across cores) ---
    def plan(core, cell, ncells, min_one):
        cnt = np.bincount(core * ncells + cell,
                          minlength=NC * ncells).reshape(NC, ncells)
        k = -(-cnt.max(0) // 128)
        k = np.maximum(k, min_one)
        off = np.zeros(ncells + 1, np.int64)
        np.cumsum(k, out=off[1:])
        return k, off * 128, int(off[-1]) * 128

    c1 = dr // S
    min1 = np.zeros(NB * G1, np.int64)
    min1[:G1] = 1  # bucket-0 cells init the Xd accumulator
    kc1, off1, T1 = plan(c1, (dc // BUCKET) * G1 + (dr % S) // 128,
                         NB * G1, min1)
    c2 = vx // S
    kc2, off2, T2 = plan(c2, eg // 128, EG, 1)
    kc4, off4, T4 = plan(c2, (vx % S) // 128, G1, 1)
    T1 = -(-T1 // 2048) * 2048  # keep /16 and /128 wrappings integral
    T2 = -(-T2 // 2048) * 2048
    T4 = -(-T4 // 2048) * 2048

    import ml_dtypes
    bf = ml_dtypes.bfloat16
    Wf = np.asarray(inputs["W_w"], np.float32)
    W1 = np.asarray(inputs["W1_w"], np.float32)
    W2 = np.asarray(inputs["W2_w"], np.float32)
    b1 = np.asarray(inputs["W1_b"], np.float32)
    b2 = np.asarray(inputs["W2_b"], np.float32)
    bb = np.asarray(inputs["W_b"], np.float32)
    U1 = (1.0 - ALPHA) * (W2[:D] @ Wf)
    U2 = (1.0 - ALPHA) * (W2[D:] @ Wf)
    U3 = ALPHA * Wf
    V1 = W1 @ U2
    c1row = b1 @ U2
    c2row = (1.0 - ALPHA) * (b2 @ Wf)

    shared = {
        "X": X,
        "U1": np.ascontiguousarray(U1).astype(bf),
        "U3": np.ascontiguousarray(U3).astype(bf),
        "V1": np.ascontiguousarray(V1).astype(bf),
        "c1_rep": np.tile(c1row, (128, 1)).astype(np.float32),
        "c2b": np.ascontiguousarray(np.stack([c2row, bb])).astype(bf),
        "cnte": _wrap128(np.bincount(eg, minlength=EP).astype(np.float32)),
        "iota16": np.ascontiguousarray(
            np.tile(np.arange(128, dtype=bf), (128, MB))),  # [128, MB*128]
        "ident": np.eye(128).astype(bf),
    }

    def fill(slots_T, cell_of_tok, kcell, offs, order, gval, dval, vval=None):
        # slots_T: total slots; cell size kcell*128; tokens sorted by `order`.
        cell = cell_of_tok[order]
        g = gval[order]
        d = dval[order]
        if len(cell):
            newc = np.empty(len(cell), bool)
            newc[0] = True
            newc[1:] = cell[1:] != cell[:-1]
            starts = np.where(newc)[0]
            idx = np.arange(len(cell))
            cell_start = np.zeros(len(cell), np.int64)
            cell_start[starts] = idx[starts]
            cell_start = np.maximum.accumulate(cell_start)
            rank = idx - cell_start
        else:
            rank = np.zeros(0, np.int64)
        slot = offs[cell] + rank
        assert len(slot) == 0 or (rank < kcell[cell] * 128).all()
        gi = np.zeros(slots_T, np.int64)
        dl = np.full(slots_T, -1.0, np.float32)
        gi[slot] = g
        dl[slot] = d
        import ml_dtypes as _md
        out = [_wrap16(gi), _wrap128(dl.astype(_md.bfloat16))]
        if vval is not None:
            vv = np.zeros(slots_T, np.float32)
            vv[slot] = vval[order]
            out.append(_wrap128(vv))
        return out

    in_maps = []
    for c in range(NC):
        lo = c * S
        m = (dr >= lo) & (dr < lo + S)
        d1 = dr[m] - lo
        c1_, v1 = dc[m], dv[m]
        b1_ = c1_ // BUCKET
        cell1 = b1_ * G1 + d1 // 128  # bucket-major cell id
        order1 = np.lexsort((c1_, cell1))   # within cell: by source column
        gi1, dl1, vv1 = fill(T1, cell1, kc1, off1, order1, c1_ - b1_ * BUCKET,
                             d1 % 128, v1)

        m2 = (vx >= lo) & (vx < lo + S)
        e2, v2 = eg[m2], vx[m2] - lo
        order2 = np.lexsort((v2, e2 // 128))  # within eg-cell: by source v
        gi2, dl2 = fill(T2, e2 // 128, kc2, off2, order2, v2, e2 % 128)
        order4 = np.lexsort((e2, v2 // 128))  # within vg-cell: by source e
        gi4, dl4 = fill(T4, v2 // 128, kc4, off4, order4, e2, v2 % 128)

        cntv = np.bincount(v2, minlength=SP).astype(np.float32)
        cnt_ones = np.ascontiguousarray(
            np.stack([cntv, np.ones(SP, np.float32)])).astype(bf)
        in_maps.append(dict(shared,
                            gidx1=gi1, drel1=dl1, val1=vv1,
                            gidx2=gi2, drel2=dl2,
                            gidx4=gi4, drel4=dl4,
                            cntv=_wrap128(cntv), cnt_ones=cnt_ones))

    meta = dict(N=N, S=S, G1=G1, SP=SP, EG=EG, EP=EP, NB=NB,
                KC1=kc1.tolist(), OFF1=off1.tolist(),
                KC2=kc2.tolist(), OFF2=off2.tolist(),
                KC4=kc4.tolist(), OFF4=off4.tolist(),
                T1=T1, T2=T2, T4=T4)
    return meta, in_maps


def _build(meta):
    from concourse import bass, bacc, tile, mybir

    f32, i16 = mybir.dt.float32, mybir.dt.int16
    bf16 = mybir.dt.bfloat16
    N, S, G1, SP, EG, EP, NB = (meta[k] for k in
                                ("N", "S", "G1", "SP", "EG", "EP", "NB"))
    T1, T2, T4 = meta["T1"], meta["T2"], meta["T4"]
    KC1, OFF1 = meta["KC1"], meta["OFF1"]
    KC2, OFF2 = meta["KC2"], meta["OFF2"]
    KC4, OFF4 = meta["KC4"], meta["OFF4"]

    nc = bacc.Bacc("TRN2", target_bir_lowering=False, debug=False,
                   num_devices=NC, num_swdge_queues=4)

    def par(name, shape, dt=f32, out=False):
        return nc.declare_dram_parameter(name, list(shape), dt, isOutput=out)

    Xp = par("X", (N, D))
    gidx1 = par("gidx1", (128, T1 // 16), i16)
    drel1 = par("drel1", (128, T1 // 128), bf16)
    val1 = par("val1", (128, T1 // 128))
    gidx2 = par("gidx2", (128, T2 // 16), i16)
    drel2 = par("drel2", (128, T2 // 128), bf16)
    gidx4 = par("gidx4", (128, T4 // 16), i16)
    drel4 = par("drel4", (128, T4 // 128), bf16)
    cntv = par("cntv", (128, G1))
    cnt_ones = par("cnt_ones", (2, SP), bf16)
    cnte = par("cnte", (128, EG))
    U1p = par("U1", (D, D), bf16)
    U3p = par("U3", (D, D), bf16)
    V1p = par("V1", (D, D), bf16)
    c1_rep = par("c1_rep", (128, D))
    c2bp = par("c2b", (2, D), bf16)
    iota16 = par("iota16", (128, MB * 128), bf16)
    ident = par("ident", (128, 128), bf16)
    OUT = par("OUT", (SP, D), out=True)

    eq = mybir.AluOpType.is_equal
    mult = mybir.AluOpType.mult
    addop = mybir.AluOpType.add
    CopyF = mybir.ActivationFunctionType.Copy

    with tile.TileContext(nc) as tc:
        with (
            tc.tile_pool(name="meta1", bufs=1) as metap,
            tc.tile_pool(name="gidxp", bufs=2) as gidxp,
            tc.tile_pool(name="gpool", bufs=16) as gpool,
            tc.tile_pool(name="mpool", bufs=8) as mpool,
            tc.tile_pool(name="psA", bufs=4, space="PSUM") as psA,
            tc.tile_pool(name="psB", bufs=2, space="PSUM") as psB,
            tc.tile_pool(name="psT", bufs=2, space="PSUM") as psT,
            tc.tile_pool(name="stage", bufs=3) as stage,
            tc.tile_pool(name="dram", bufs=1, space="DRAM") as dram,
        ):
            # --- resident metadata ---
            def load(ap_param, shape, nm, dt=f32, pool=metap):
                t = pool.tile(list(shape), dt, name=nm, tag=nm)
                nc.scalar.dma_start(t[:], ap_param[:])
                return t

            iota_t = load(iota16, (128, MB, 128), "iota_t", dt=bf16)
            ident_t = load(ident, (128, 128), "ident_t", dt=bf16)
            u1_t = load(U1p, (D, D), "u1_t", dt=bf16)
            u3_t = load(U3p, (D, D), "u3_t", dt=bf16)
            v1_t = load(V1p, (D, D), "v1_t", dt=bf16)
            c1_t = load(c1_rep, (128, D), "c1_t")
            c2b_t = load(c2bp, (2, D), "c2b_t", dt=bf16)
            cntv_t = load(cntv, (128, G1), "cntv_t")
            cnto_t = load(cnt_ones, (2, SP), "cnto_t", dt=bf16)
            cnte_t = load(cnte, (128, EG), "cnte_t")
            drel1_t = load(drel1, (128, T1 // 128), "drel1_t", dt=bf16)
            val1_t = load(val1, (128, T1 // 128), "val1_t")
            drel2_t = load(drel2, (128, T2 // 128), "drel2_t", dt=bf16)
            drel4_t = load(drel4, (128, T4 // 128), "drel4_t", dt=bf16)

            Xd_sb = metap.tile([128, G1, D], f32)     # wrapped node shard
            XdB_sb = metap.tile([128, G1, D], bf16)
            xdT_all = metap.tile([D, G1, 128], bf16)  # per-group Xd^T
            Xd_hbm = dram.tile([SP, 2 * D], bf16)
            Z_hbm = dram.tile([EP, 2 * D], bf16)
            EGH = (EG // 2) // 4 * 4  # first-half edge groups (AllReduce split)
            A_part1 = dram.tile([EGH * D, 128], bf16)
            A_part2 = dram.tile([(EG - EGH) * D, 128], bf16)
            A_full1 = dram.tile([EGH * D, 128], bf16, addr_space="Shared")
            A_full2 = dram.tile([(EG - EGH) * D, 128], bf16,
                                addr_space="Shared")
            qctr = [0]

            def sparse_step(gidx_par, gidx_cols, drel_t, val_t, srcs,
                            kcells, offs, evac, src_dt, src_cols, swap,
                            open_group=False, cell_lo=0, cell_hi=None):
                """srcs: (src_ap, cell_lo, cell_hi) bucket streams covering
                cells [lo, hi); slot spans from offs. swap: rows stationary,
                one-hot moving -> psum [64,128]. open_group: leave the PSUM
                matmul group open (evac closes it)."""
                if cell_hi is None:
                    cell_hi = len(kcells)
                s_lo, s_hi = offs[cell_lo], offs[cell_hi]
                gidx_t = gidxp.tile([128, (s_hi - s_lo) // 16], i16,
                                    tag="gidx")
                nc.scalar.dma_start(gidx_t[:],
                                    gidx_par[:, s_lo // 16:s_hi // 16])
                tile_src = {}
                for src_ap, c_lo, c_hi in srcs:
                    base, end = offs[c_lo], offs[c_hi]
                    off = 0
                    L = end - base
                    while off < L:
                        n = min(CHUNK, L - off)
                        cols = n // 128
                        gt = gpool.tile([128, CHUNK // 128, src_cols], src_dt,
                                        tag="g")
                        nc.gpsimd.dma_gather(
                            gt[:, :cols, :], src_ap,
                            gidx_t[:, (base + off - s_lo) // 16:
                                   (base + off + n - s_lo) // 16],
                            n, n, src_cols, queue_num=qctr[0] % 4)
                        qctr[0] += 1
                        if val_t is not None:
                            g2 = gpool.tile([128, CHUNK // 128, D], bf16,
                                            tag="g2")
                            vs = val_t[:, (base + off) // 128:
                                       (base + off) // 128 + cols]
                            nc.vector.tensor_mul(
                                g2[:, :cols, :], gt[:, :cols, :D],
                                vs.unsqueeze(2).broadcast_to([128, cols, D]))
                            src_t = g2
                        else:
                            src_t = gt
                        for i in range(cols):
                            tile_src[(base + off) // 128 + i] = (src_t, i)
                        off += n
                ntiles = offs[len(kcells)] // 128
                mb_next = 0
                m_buf = None
                mb_base = 0
                for cell in range(cell_lo, cell_hi):
                    kt = kcells[cell]
                    if kt == 0:
                        continue
                    cur_full = psA.tile([128, 128], f32, tag="acc")
                    cur = cur_full[0:D, :] if swap else cur_full[:, 0:D]
                    t0 = offs[cell] // 128
                    for i in range(kt):
                        t = t0 + i
                        if t >= mb_next:
                            k = min(MB, ntiles - t)
                            m_buf = mpool.tile([128, MB, 128],
                                              mybir.dt.float8e4, tag="m")
                            db = drel_t[:, t:t + k].unsqueeze(2).broadcast_to(
                                [128, k, 128])
                            nc.vector.tensor_tensor(m_buf[:, :k, :],
                                                    iota_t[:, :k, :], db, eq)
                            mb_base, mb_next = t, t + k
                        gt, col = tile_src[t]
                        last = (i == kt - 1) and not open_group
                        if swap:
                            nc.tensor.matmul(cur[:], gt[:, col, :D],
                                             m_buf[:, t - mb_base, :],
                                             start=(i == 0), stop=last)
                        else:
                            nc.tensor.matmul(cur[:], m_buf[:, t - mb_base, :],
                                             gt[:, col, :D],
                                             start=(i == 0), stop=last)
                    evac(cell, cur)

            # ---- step 1: diffusion into Xd ----
            srcs1 = []
            for b in range(NB):
                rows = min(BUCKET, N - b * BUCKET)
                srcs1.append((Xp[b * BUCKET:b * BUCKET + rows, :],
                              b * G1, (b + 1) * G1))

            def evac1(cellidx, psum):
                b, g = divmod(cellidx, G1)
                if b == 0:
                    nc.scalar.activation(Xd_sb[:, g, :], psum[:], CopyF)
                else:
                    nc.vector.tensor_add(Xd_sb[:, g, :], Xd_sb[:, g, :],
                                         psum[:])
                if b == NB - 1:
                    nc.scalar.activation(XdB_sb[:, g, :], Xd_sb[:, g, :],
                                         CopyF)

            sparse_step(gidx1, T1 // 16, drel1_t, val1_t, srcs1, KC1, OFF1,
                        evac1, f32, D, swap=False)

            # Xd wrapped -> row-major bf16 HBM table (step-2 gather source);
            # chunked so early groups upload while late cells still compute
            WCH = 14
            for g0 in range(0, G1, WCH):
                gn = min(WCH, G1 - g0)
                nc.sync.dma_start(
                    Xd_hbm[g0 * 128:(g0 + gn) * 128, :D]
                    .rearrange("(g p) f -> p g f", p=128),
                    XdB_sb[:, g0:g0 + gn, :])

            # ---- step 2: A^T[e] partials (rows stationary, one-hot moving) --
            ev2 = {}

            def evac2(g, psum):
                b = g % 4
                if b == 0:
                    ev2["t"] = stage.tile([D, 4, 128], bf16, tag="ev2",
                                          name="ev2t")
                    ev2["g0"] = g
                nc.scalar.activation(ev2["t"][:, b, :], psum[:], CopyF)
                if b == 3 or g == EG - 1:
                    nb = b + 1
                    g0 = ev2["g0"]
                    dst, base = ((A_part1, 0) if g0 < EGH
                                 else (A_part2, EGH))
                    nc.sync.dma_start(
                        dst[(g0 - base) * D:(g0 - base + nb) * D, :]
                        .rearrange("(b p) f -> p b f", p=D),
                        ev2["t"][:, :nb, :])

            sparse_step(gidx2, T2 // 16, drel2_t, None,
                        [(Xd_hbm[:, :], 0, EGH)], KC2, OFF2, evac2,
                        bf16, 2 * D, swap=True, cell_lo=0, cell_hi=EGH)
            # AR of the first half runs on the CC cores while the second
            # half's gathers stream
            nc.gpsimd.collective_compute(
                "AllReduce", addop,
                replica_groups=[list(range(NC))],
                ins=[A_part1.opt()], outs=[A_full1.opt()])
            sparse_step(gidx2, T2 // 16, drel2_t, None,
                        [(Xd_hbm[:, :], EGH, EG)], KC2, OFF2, evac2,
                        bf16, 2 * D, swap=True, cell_lo=EGH, cell_hi=EG)

            # ---- AllReduce A^T (second half) ----
            nc.gpsimd.collective_compute(
                "AllReduce", addop,
                replica_groups=[list(range(NC))],
                ins=[A_part2.opt()], outs=[A_full2.opt()])

            # per-group Xd^T for step 5 (PE transpose, Act evac) -- issued
            # here so it overlaps step-2 gathers / the collectives
            for g in range(G1):
                pT = psT.tile([D, 128], bf16, tag="t")
                nc.tensor.transpose(pT[:], XdB_sb[:, g, :], ident_t[:])
                nc.scalar.activation(xdT_all[:, g, :], pT[:], CopyF)

            # ---- step 3: Z = A @ V1 + cnt_e x c1 (no transposes) ----
            ev3 = {}
            a4 = None
            for g in range(EG):
                if g % 4 == 0:
                    nb4 = min(4, EG - g)
                    a4 = stage.tile([D, 4, 128], bf16, tag="a", name="a4")
                    src_t, sbase = ((A_full1, 0) if g < EGH
                                    else (A_full2, EGH))
                    nc.scalar.dma_start(
                        a4[:, :nb4, :],
                        src_t[(g - sbase) * D:(g - sbase + nb4) * D, :]
                        .rearrange("(b p) f -> p b f", p=D))
                p2 = psB.tile([128, D], f32, tag="acc1")
                nc.tensor.matmul(p2[:], a4[:, g % 4, :], v1_t[:],
                                 start=True, stop=True)
                b4 = g % 4
                if b4 == 0:
                    ev3["t"] = stage.tile([128, 4, D], bf16, tag="ev3",
                                          name="ev3t")
                    ev3["g0"] = g
                nc.vector.scalar_tensor_tensor(
                    ev3["t"][:, b4, :], c1_t[:], cnte_t[:, g:g + 1], p2[:],
                    mult, addop)
                if b4 == 3 or g == EG - 1:
                    nb = b4 + 1
                    nc.sync.dma_start(
                        Z_hbm[ev3["g0"] * 128:(ev3["g0"] + nb) * 128, :D]
                        .rearrange("(b p) f -> p b f", p=128),
                        ev3["t"][:, :nb, :])

            # ---- step 4 + step 5 fused ----
            otb = [None]

            def evac4(g, psum):
                # append:  + Xd[g]@U3  + cnt_v x c2 + 1 x b   (then stop)
                nc.tensor.matmul(psum[:], xdT_all[:, g, :], u3_t[:],
                                 start=False, stop=False)
                nc.tensor.matmul(psum[:], cnto_t[:, g * 128:(g + 1) * 128],
                                 c2b_t[:], start=False, stop=True)
                p1 = psB.tile([128, D], f32, tag="acc1")
                nc.tensor.matmul(p1[:], xdT_all[:, g, :], u1_t[:],
                                 start=True, stop=True)
                t1 = stage.tile([128, D], f32, tag="t1")
                nc.scalar.activation(t1[:], p1[:], CopyF)
                b4 = g % 4
                if b4 == 0:
                    otb[0] = stage.tile([128, 4, D], f32, tag="otb",
                                        name="otb")
                nc.vector.scalar_tensor_tensor(
                    otb[0][:, b4, :], t1[:], cntv_t[:, g:g + 1], psum[:],
                    mult, addop)
                if b4 == 3 or g == G1 - 1:
                    nb = b4 + 1
                    nc.sync.dma_start(
                        OUT[(g - nb + 1) * 128:(g + 1) * 128, :]
                        .rearrange("(b p) f -> p b f", p=128),
                        otb[0][:, :nb, :])

            sparse_step(gidx4, T4 // 16, drel4_t, None,
                        [(Z_hbm[:, :], 0, G1)], KC4, OFF4, evac4,
                        bf16, 2 * D, swap=False, open_group=True)

    nc.compile()
    return nc


def _run(inputs, n_edges, sim=False):
    meta, in_maps = _prep(inputs, n_edges)
    nc = _build(meta)
    S, SP = meta["S"], meta["SP"]
    if sim:
        from concourse import bass_interp
        ms = bass_interp.MultiCoreSim(nc, NC, require_finite=False,
                                      require_nnan=False)
        for c in range(NC):
            for k, v in in_maps[c].items():
                ms.cores[c].tensor(k)[:] = v
        ms.simulate()
        outs = [np.array(ms.cores[c].mem_tensor("OUT")).reshape(SP, D)
                for c in range(NC)]
    else:
        from concourse.bass_utils import run_bass_kernel_spmd
        res = run_bass_kernel_spmd(nc, in_maps, list(range(NC)),
                                   trace=TRACE)
        global LAST_EXEC_NS, LAST_RESULTS
        LAST_EXEC_NS = res.exec_time_ns
        LAST_RESULTS = res
        outs = [res.results[c]["OUT"] for c in range(NC)]
    return np.concatenate([o[:S] for o in outs], axis=0).astype(np.float32)


def kernel(**inputs):
    return _run(inputs, 25000, sim=False)


# revision 14
# speedup vs baseline: 1.2834x; 1.2834x over previous
"""EquivSetConv (hypergraph message passing) Trainium2 Bass kernel.

Math (reference):
  Xd = segment_sum(dif_vals * X[dif_cols], dif_rows, N)
  Xe = segment_sum((Xd@W1+b1)[vertex], edges, E)
  Xv = segment_sum(concat(Xd[vertex], Xe[edges]) @ W2 + b2, vertex, N)
  out = ((1-a)*Xv + a*Xd) @ W + b

Algebraic reassociation (exact up to fp reassociation), with
U1=(1-a)W2top@W, U2=(1-a)W2bot@W, U3=aW, V1=W1@U2, c1=b1@U2,
c2=(1-a)(b2@W):
  A[e]  = segment_sum(Xd[vertex], edges, E)
  Z     = A @ V1 + cnt_e x c1          (= Xe @ U2)
  B'[v] = segment_sum(Z[edges], vertex, N)
  out   = cnt_v o (Xd@U1) + Xd@U3 + B' + cnt_v x c2 + b

Distribution: nodes sharded 8 ways by row range; incidence lists bucketed by
destination core; the only collective is an AllReduce of the per-core partial
A^T [EG*64,128] bf16. Segment sums run as one-hot matmul accumulation in PSUM
over 128-destination groups; gathers use dma_gather (int16 idx, <=1024/call).
Step 2 uses swapped matmul operands (rows stationary, one-hot moving) so A is
produced transposed and step 3 needs no transposes. Step 5 is fused into
step 4's PSUM accumulation groups.
"""
import sys
import numpy as np

sys.path.insert(0, "/opt/trn_rl_repo")

D = 64
NC = 8
CHUNK = 1024        # dma_gather per-call token cap
MB = 16             # one-hot tiles built per DVE op
ALPHA = 0.5
BUCKET = 32768      # int16 gather index range
TRACE = False
LAST_EXEC_NS = None
LAST_RESULTS = None


def _wrap16(a):
    a = np.asarray(a, np.int16)
    return np.tile(a.reshape(-1, 16).T, (8, 1))  # [128, T/16]


def _wrap128(a):
    return np.ascontiguousarray(np.asarray(a).reshape(-1, 128).T)  # [128, T/128]


def _prep(inputs, n_edges):
    X = np.ascontiguousarray(np.asarray(inputs["X"], np.float32))
    N = X.shape[0]
    assert N % NC == 0
    S = N // NC
    G1 = -(-S // 128)          # node groups per core
    SP = G1 * 128
    EG = -(-n_edges // 128)    # edge groups
    EP = EG * 128
    NB = -(-N // BUCKET)

    dr = np.asarray(inputs["dif_rows"], np.int64)
    dc = np.asarray(inputs["dif_cols"], np.int64)
    dv = np.asarray(inputs["dif_vals"], np.float32)
    vx = np.asarray(inputs["vertex"], np.int64)
    eg = np.asarray(inputs["edges"], np.int64)
    assert eg.max() < n_edges and vx.max() < N and dr.max() < N and dc.max() < N

    # --- per-cell tile plans (max fill over cores; uniform across cores) ---
    def plan(core, cell, ncells, min_one):
        cnt = np.bincount(core * ncells + cell,
                          minlength=NC * ncells).reshape(NC, ncells)
        k = -(-cnt.max(0) // 128)
        k = np.maximum(k, min_one)
        off = np.zeros(ncells + 1, np.int64)
        np.cumsum(k, out=off[1:])
        return k, off * 128, int(off[-1]) * 128

    c1 = dr // S
    min1 = np.zeros(NB * G1, np.int64)
    min1[:G1] = 1  # bucket-0 cells init the Xd accumulator
    kc1, off1, T1 = plan(c1, (dc // BUCKET) * G1 + (dr % S) // 128,
                         NB * G1, min1)
    c2 = vx // S
    kc2, off2, T2 = plan(c2, eg // 128, EG, 1)
    kc4, off4, T4 = plan(c2, (vx % S) // 128, G1, 1)
    T1 = -(-T1 // 2048) * 2048  # keep /16 and /128 wrappings integral
    T2 = -(-T2 // 2048) * 2048
    T4 = -(-T4 // 2048) * 2048

    import ml_dtypes
    bf = ml_dtypes.bfloat16
    Wf = np.asarray(inputs["W_w"], np.float32)
    W1 = np.asarray(inputs["W1_w"], np.float32)
    W2 = np.asarray(inputs["W2_w"], np.float32)
    b1 = np.asarray(inputs["W1_b"], np.float32)
    b2 = np.asarray(inputs["W2_b"], np.float32)
    bb = np.asarray(inputs["W_b"], np.float32)
    U1 = (1.0 - ALPHA) * (W2[:D] @ Wf)
    U2 = (1.0 - ALPHA) * (W2[D:] @ Wf)
    U3 = ALPHA * Wf
    V1 = W1 @ U2
    c1row = b1 @ U2
    c2row = (1.0 - ALPHA) * (b2 @ Wf)

    shared = {
        "X": X,
        "U1": np.ascontiguousarray(U1).astype(bf),
        "U3": np.ascontiguousarray(U3).astype(bf),
        "V1": np.ascontiguousarray(V1).astype(bf),
        "c1_rep": np.tile(c1row, (128, 1)).astype(np.float32),
        "c2b": np.ascontiguousarray(np.stack([c2row, bb])).astype(bf),
        "cnte": _wrap128(np.bincount(eg, minlength=EP).astype(np.float32)),
        "iota16": np.ascontiguousarray(
            np.tile(np.arange(128, dtype=bf), (128, MB))),  # [128, MB*128]
        "ident": np.eye(128).astype(bf),
    }

    def fill(slots_T, cell_of_tok, kcell, offs, order, gval, dval, vval=None):
        # slots_T: total slots; cell size kcell*128; tokens sorted by `order`.
        cell = cell_of_tok[order]
        g = gval[order]
        d = dval[order]
        if len(cell):
            newc = np.empty(len(cell), bool)
            newc[0] = True
            newc[1:] = cell[1:] != cell[:-1]
            starts = np.where(newc)[0]
            idx = np.arange(len(cell))
            cell_start = np.zeros(len(cell), np.int64)
            cell_start[starts] = idx[starts]
            cell_start = np.maximum.accumulate(cell_start)
            rank = idx - cell_start
        else:
            rank = np.zeros(0, np.int64)
        slot = offs[cell] + rank
        assert len(slot) == 0 or (rank < kcell[cell] * 128).all()
        gi = np.zeros(slots_T, np.int64)
        dl = np.full(slots_T, -1.0, np.float32)
        gi[slot] = g
        dl[slot] = d
        import ml_dtypes as _md
        out = [_wrap16(gi), _wrap128(dl.astype(_md.bfloat16))]
        if vval is not None:
            vv = np.zeros(slots_T, np.float32)
            vv[slot] = vval[order]
            out.append(_wrap128(vv))
        return out

    in_maps = []
    for c in range(NC):
        lo = c * S
        m = (dr >= lo) & (dr < lo + S)
        d1 = dr[m] - lo
        c1_, v1 = dc[m], dv[m]
        b1_ = c1_ // BUCKET
        cell1 = b1_ * G1 + d1 // 128  # bucket-major cell id
        order1 = np.lexsort((c1_, cell1))   # within cell: by source column
        gi1, dl1, vv1 = fill(T1, cell1, kc1, off1, order1, c1_ - b1_ * BUCKET,
                             d1 % 128, v1)

        m2 = (vx >= lo) & (vx < lo + S)
        e2, v2 = eg[m2], vx[m2] - lo
        order2 = np.lexsort((v2, e2 // 128))  # within eg-cell: by source v
        gi2, dl2 = fill(T2, e2 // 128, kc2, off2, order2, v2, e2 % 128)
        order4 = np.lexsort((e2, v2 // 128))  # within vg-cell: by source e
        gi4, dl4 = fill(T4, v2 // 128, kc4, off4, order4, e2, v2 % 128)

        cntv = np.bincount(v2, minlength=SP).astype(np.float32)
        cnt_ones = np.ascontiguousarray(
            np.stack([cntv, np.ones(SP, np.float32)])).astype(bf)
        in_maps.append(dict(shared,
                            gidx1=gi1, drel1=dl1, val1=vv1,
                            gidx2=gi2, drel2=dl2,
                            gidx4=gi4, drel4=dl4,
                            cntv=_wrap128(cntv), cnt_ones=cnt_ones))

    meta = dict(N=N, S=S, G1=G1, SP=SP, EG=EG, EP=EP, NB=NB,
                KC1=kc1.tolist(), OFF1=off1.tolist(),
                KC2=kc2.tolist(), OFF2=off2.tolist(),
                KC4=kc4.tolist(), OFF4=off4.tolist(),
                T1=T1, T2=T2, T4=T4)
    return meta, in_maps


def _build(meta):
    from concourse import bass, bacc, tile, mybir

    f32, i16 = mybir.dt.float32, mybir.dt.int16
    bf16 = mybir.dt.bfloat16
    N, S, G1, SP, EG, EP, NB = (meta[k] for k in
                                ("N", "S", "G1", "SP", "EG", "EP", "NB"))
    T1, T2, T4 = meta["T1"], meta["T2"], meta["T4"]
    KC1, OFF1 = meta["KC1"], meta["OFF1"]
    KC2, OFF2 = meta["KC2"], meta["OFF2"]
    KC4, OFF4 = meta["KC4"], meta["OFF4"]

    nc = bacc.Bacc("TRN2", target_bir_lowering=False, debug=False,
                   num_devices=NC, num_swdge_queues=4)

    def par(name, shape, dt=f32, out=False):
        return nc.declare_dram_parameter(name, list(shape), dt, isOutput=out)

    Xp = par("X", (N, D))
    gidx1 = par("gidx1", (128, T1 // 16), i16)
    drel1 = par("drel1", (128, T1 // 128), bf16)
    val1 = par("val1", (128, T1 // 128))
    gidx2 = par("gidx2", (128, T2 // 16), i16)
    drel2 = par("drel2", (128, T2 // 128), bf16)
    gidx4 = par("gidx4", (128, T4 // 16), i16)
    drel4 = par("drel4", (128, T4 // 128), bf16)
    cntv = par("cntv", (128, G1))
    cnt_ones = par("cnt_ones", (2, SP), bf16)
    cnte = par("cnte", (128, EG))
    U1p = par("U1", (D, D), bf16)
    U3p = par("U3", (D, D), bf16)
    V1p = par("V1", (D, D), bf16)
    c1_rep = par("c1_rep", (128, D))
    c2bp = par("c2b", (2, D), bf16)
    iota16 = par("iota16", (128, MB * 128), bf16)
    ident = par("ident", (128, 128), bf16)
    OUT = par("OUT", (SP, D), out=True)

    eq = mybir.AluOpType.is_equal
    mult = mybir.AluOpType.mult
    addop = mybir.AluOpType.add
    CopyF = mybir.ActivationFunctionType.Copy

    with tile.TileContext(nc) as tc:
        with (
            tc.tile_pool(name="meta1", bufs=1) as metap,
            tc.tile_pool(name="gidxp", bufs=2) as gidxp,
            tc.tile_pool(name="gpool", bufs=12) as gpool,
            tc.tile_pool(name="mpool", bufs=4) as mpool,
            tc.tile_pool(name="psA", bufs=3, space="PSUM") as psA,
            tc.tile_pool(name="psB", bufs=2, space="PSUM") as psB,
            tc.tile_pool(name="psT", bufs=2, space="PSUM") as psT,
            tc.tile_pool(name="stage", bufs=3) as stage,
            tc.tile_pool(name="dram", bufs=1, space="DRAM") as dram,
        ):
            # --- resident metadata ---
            def load(ap_param, shape, nm, dt=f32, pool=metap):
                t = pool.tile(list(shape), dt, name=nm, tag=nm)
                nc.scalar.dma_start(t[:], ap_param[:])
                return t

            iota_t = load(iota16, (128, MB, 128), "iota_t", dt=bf16)
            ident_t = load(ident, (128, 128), "ident_t", dt=bf16)
            u1_t = load(U1p, (D, D), "u1_t", dt=bf16)
            u3_t = load(U3p, (D, D), "u3_t", dt=bf16)
            v1_t = load(V1p, (D, D), "v1_t", dt=bf16)
            c1_t = load(c1_rep, (128, D), "c1_t")
            c2b_t = load(c2bp, (2, D), "c2b_t", dt=bf16)
            cntv_t = load(cntv, (128, G1), "cntv_t")
            cnto_t = load(cnt_ones, (2, SP), "cnto_t", dt=bf16)
            cnte_t = load(cnte, (128, EG), "cnte_t")
            drel1_t = load(drel1, (128, T1 // 128), "drel1_t", dt=bf16)
            val1_t = load(val1, (128, T1 // 128), "val1_t")
            drel2_t = load(drel2, (128, T2 // 128), "drel2_t", dt=bf16)
            drel4_t = load(drel4, (128, T4 // 128), "drel4_t", dt=bf16)

            Xd_sb = metap.tile([128, G1, D], f32)     # wrapped node shard
            XdB_sb = metap.tile([128, G1, D], bf16)
            xdT_all = metap.tile([D, G1, 128], bf16)  # per-group Xd^T
            Xd_hbm = dram.tile([SP, 2 * D], bf16)
            Z_hbm = dram.tile([EP, 2 * D], bf16)
            EGH = (EG // 2) // 4 * 4  # first-half edge groups (AllReduce split)
            A_part1 = dram.tile([EGH * D, 128], bf16)
            A_part2 = dram.tile([(EG - EGH) * D, 128], bf16)
            A_full1 = dram.tile([EGH * D, 128], bf16, addr_space="Shared")
            A_full2 = dram.tile([(EG - EGH) * D, 128], bf16,
                                addr_space="Shared")
            qctr = [0]

            def sparse_step(gidx_par, gidx_cols, drel_t, val_t, srcs,
                            kcells, offs, evac, src_dt, src_cols, swap,
                            open_group=False, cell_lo=0, cell_hi=None):
                """srcs: (src_ap, cell_lo, cell_hi) bucket streams covering
                cells [lo, hi); slot spans from offs. swap: rows stationary,
                one-hot moving -> psum [64,128]. open_group: leave the PSUM
                matmul group open (evac closes it)."""
                if cell_hi is None:
                    cell_hi = len(kcells)
                s_lo, s_hi = offs[cell_lo], offs[cell_hi]
                gidx_t = gidxp.tile([128, (s_hi - s_lo) // 16], i16,
                                    tag="gidx")
                nc.scalar.dma_start(gidx_t[:],
                                    gidx_par[:, s_lo // 16:s_hi // 16])
                tile_src = {}
                for src_ap, c_lo, c_hi in srcs:
                    base, end = offs[c_lo], offs[c_hi]
                    off = 0
                    L = end - base
                    while off < L:
                        n = min(CHUNK, L - off)
                        cols = n // 128
                        gt = gpool.tile([128, CHUNK // 128, src_cols], src_dt,
                                        tag="g")
                        nc.gpsimd.dma_gather(
                            gt[:, :cols, :], src_ap,
                            gidx_t[:, (base + off - s_lo) // 16:
                                   (base + off + n - s_lo) // 16],
                            n, n, src_cols, queue_num=qctr[0] % 4)
                        qctr[0] += 1
                        if val_t is not None:
                            g2 = gpool.tile([128, CHUNK // 128, D], bf16,
                                            tag="g2")
                            vs = val_t[:, (base + off) // 128:
                                       (base + off) // 128 + cols]
                            nc.vector.tensor_mul(
                                g2[:, :cols, :], gt[:, :cols, :D],
                                vs.unsqueeze(2).broadcast_to([128, cols, D]))
                            src_t = g2
                        else:
                            src_t = gt
                        for i in range(cols):
                            tile_src[(base + off) // 128 + i] = (src_t, i)
                        off += n
                ntiles = offs[len(kcells)] // 128
                mb_next = 0
                m_buf = None
                mb_base = 0
                for cell in range(cell_lo, cell_hi):
                    kt = kcells[cell]
                    if kt == 0:
                        continue
                    cur_full = psA.tile([128, 128], f32, tag="acc")
                    cur = cur_full[0:D, :] if swap else cur_full[:, 0:D]
                    t0 = offs[cell] // 128
                    for i in range(kt):
                        t = t0 + i
                        if t >= mb_next:
                            k = min(MB, ntiles - t)
                            m_buf = mpool.tile([128, MB, 128],
                                              mybir.dt.float8e4, tag="m")
                            db = drel_t[:, t:t + k].unsqueeze(2).broadcast_to(
                                [128, k, 128])
                            nc.vector.tensor_tensor(m_buf[:, :k, :],
                                                    iota_t[:, :k, :], db, eq)
                            mb_base, mb_next = t, t + k
                        gt, col = tile_src[t]
                        last = (i == kt - 1) and not open_group
                        if swap:
                            nc.tensor.matmul(cur[:], gt[:, col, :D],
                                             m_buf[:, t - mb_base, :],
                                             start=(i == 0), stop=last)
                        else:
                            nc.tensor.matmul(cur[:], m_buf[:, t - mb_base, :],
                                             gt[:, col, :D],
                                             start=(i == 0), stop=last)
                    evac(cell, cur)

            # ---- step 1: diffusion into Xd ----
            srcs1 = []
            for b in range(NB):
                rows = min(BUCKET, N - b * BUCKET)
                srcs1.append((Xp[b * BUCKET:b * BUCKET + rows, :],
                              b * G1, (b + 1) * G1))

            def evac1(cellidx, psum):
                b, g = divmod(cellidx, G1)
                if b == 0:
                    nc.scalar.activation(Xd_sb[:, g, :], psum[:], CopyF)
                else:
                    nc.vector.tensor_add(Xd_sb[:, g, :], Xd_sb[:, g, :],
                                         psum[:])
                if b == NB - 1:
                    nc.scalar.activation(XdB_sb[:, g, :], Xd_sb[:, g, :],
                                         CopyF)

            sparse_step(gidx1, T1 // 16, drel1_t, val1_t, srcs1, KC1, OFF1,
                        evac1, f32, D, swap=False)

            # Xd wrapped -> row-major bf16 HBM table (step-2 gather source);
            # chunked so early groups upload while late cells still compute
            WCH = 14
            for g0 in range(0, G1, WCH):
                gn = min(WCH, G1 - g0)
                nc.sync.dma_start(
                    Xd_hbm[g0 * 128:(g0 + gn) * 128, :D]
                    .rearrange("(g p) f -> p g f", p=128),
                    XdB_sb[:, g0:g0 + gn, :])

            # ---- step 2: A^T[e] partials (rows stationary, one-hot moving) --
            ev2 = {}

            def evac2(g, psum):
                b = g % 4
                if b == 0:
                    ev2["t"] = stage.tile([D, 4, 128], bf16, tag="ev2",
                                          name="ev2t")
                    ev2["g0"] = g
                nc.scalar.activation(ev2["t"][:, b, :], psum[:], CopyF)
                if b == 3 or g == EG - 1:
                    nb = b + 1
                    g0 = ev2["g0"]
                    dst, base = ((A_part1, 0) if g0 < EGH
                                 else (A_part2, EGH))
                    nc.sync.dma_start(
                        dst[(g0 - base) * D:(g0 - base + nb) * D, :]
                        .rearrange("(b p) f -> p b f", p=D),
                        ev2["t"][:, :nb, :])

            sparse_step(gidx2, T2 // 16, drel2_t, None,
                        [(Xd_hbm[:, :], 0, EGH)], KC2, OFF2, evac2,
                        bf16, 2 * D, swap=True, cell_lo=0, cell_hi=EGH)
            # AR of the first half runs on the CC cores while the second
            # half's gathers stream
            nc.gpsimd.collective_compute(
                "AllReduce", addop,
                replica_groups=[list(range(NC))],
                ins=[A_part1.opt()], outs=[A_full1.opt()])
            sparse_step(gidx2, T2 // 16, drel2_t, None,
                        [(Xd_hbm[:, :], EGH, EG)], KC2, OFF2, evac2,
                        bf16, 2 * D, swap=True, cell_lo=EGH, cell_hi=EG)

            # ---- AllReduce A^T (second half) ----
            nc.gpsimd.collective_compute(
                "AllReduce", addop,
                replica_groups=[list(range(NC))],
                ins=[A_part2.opt()], outs=[A_full2.opt()])

            # per-group Xd^T for step 5 (PE transpose, Act evac) -- issued
            # here so it overlaps step-2 gathers / the collectives
            for g in range(G1):
                pT = psT.tile([D, 128], bf16, tag="t")
                nc.tensor.transpose(pT[:], XdB_sb[:, g, :], ident_t[:])
                nc.scalar.activation(xdT_all[:, g, :], pT[:], CopyF)

            # ---- step 3: Z = A @ V1 + cnt_e x c1 (no transposes) ----
            ev3 = {}
            a4 = None
            for g in range(EG):
                if g % 4 == 0:
                    nb4 = min(4, EG - g)
                    a4 = stage.tile([D, 4, 128], bf16, tag="a", name="a4")
                    src_t, sbase = ((A_full1, 0) if g < EGH
                                    else (A_full2, EGH))
                    nc.scalar.dma_start(
                        a4[:, :nb4, :],
                        src_t[(g - sbase) * D:(g - sbase + nb4) * D, :]
                        .rearrange("(b p) f -> p b f", p=D))
                p2 = psB.tile([128, D], f32, tag="acc1")
                nc.tensor.matmul(p2[:], a4[:, g % 4, :], v1_t[:],
                                 start=True, stop=True)
                b4 = g % 4
                if b4 == 0:
                    ev3["t"] = stage.tile([128, 4, D], bf16, tag="ev3",
                                          name="ev3t")
                    ev3["g0"] = g
                nc.vector.scalar_tensor_tensor(
                    ev3["t"][:, b4, :], c1_t[:], cnte_t[:, g:g + 1], p2[:],
                    mult, addop)
                if b4 == 3 or g == EG - 1:
                    nb = b4 + 1
                    nc.sync.dma_start(
                        Z_hbm[ev3["g0"] * 128:(ev3["g0"] + nb) * 128, :D]
                        .rearrange("(b p) f -> p b f", p=128),
                        ev3["t"][:, :nb, :])

            # ---- step 4 + step 5 fused ----
            otb = [None]

            def evac4(g, psum):
                # append:  + Xd[g]@U3  + cnt_v x c2 + 1 x b   (then stop)
                nc.tensor.matmul(psum[:], xdT_all[:, g, :], u3_t[:],
                                 start=False, stop=False)
                nc.tensor.matmul(psum[:], cnto_t[:, g * 128:(g + 1) * 128],
                                 c2b_t[:], start=False, stop=True)
                p1 = psB.tile([128, D], f32, tag="acc1")
                nc.tensor.matmul(p1[:], xdT_all[:, g, :], u1_t[:],
                                 start=True, stop=True)
                t1 = stage.tile([128, D], f32, tag="t1")
                nc.scalar.activation(t1[:], p1[:], CopyF)
                b4 = g % 4
                if b4 == 0:
                    otb[0] = stage.tile([128, 4, D], f32, tag="otb",
                                        name="otb")
                nc.vector.scalar_tensor_tensor(
                    otb[0][:, b4, :], t1[:], cntv_t[:, g:g + 1], psum[:],
                    mult, addop)
                if b4 == 3 or g == G1 - 1:
                    nb = b4 + 1
                    nc.sync.dma_start(
                        OUT[(g - nb + 1) * 128:(g + 1) * 128, :]
                        .rearrange("(b p) f -> p b f", p=128),
                        otb[0][:, :nb, :])

            sparse_step(gidx4, T4 // 16, drel4_t, None,
                        [(Z_hbm[:, :], 0, G1)], KC4, OFF4, evac4,
                        bf16, 2 * D, swap=False, open_group=True)

    nc.compile()
    return nc


def _run(inputs, n_edges, sim=False):
    meta, in_maps = _prep(inputs, n_edges)
    nc = _build(meta)
    S, SP = meta["S"], meta["SP"]
    if sim:
        from concourse import bass_interp
        ms = bass_interp.MultiCoreSim(nc, NC, require_finite=False,
                                      require_nnan=False)
        for c in range(NC):
            for k, v in in_maps[c].items():
                ms.cores[c].tensor(k)[:] = v
        ms.simulate()
        outs = [np.array(ms.cores[c].mem_tensor("OUT")).reshape(SP, D)
                for c in range(NC)]
    else:
        from concourse.bass_utils import run_bass_kernel_spmd
        res = run_bass_kernel_spmd(nc, in_maps, list(range(NC)),
                                   trace=TRACE)
        global LAST_EXEC_NS, LAST_RESULTS
        LAST_EXEC_NS = res.exec_time_ns
        LAST_RESULTS = res
        outs = [res.results[c]["OUT"] for c in range(NC)]
    return np.concatenate([o[:S] for o in outs], axis=0).astype(np.float32)


def kernel(**inputs):
    return _run(inputs, 25000, sim=False)


# revision 15
# speedup vs baseline: 1.3054x; 1.0171x over previous
"""EquivSetConv (hypergraph message passing) Trainium2 Bass kernel.

Math (reference):
  Xd = segment_sum(dif_vals * X[dif_cols], dif_rows, N)
  Xe = segment_sum((Xd@W1+b1)[vertex], edges, E)
  Xv = segment_sum(concat(Xd[vertex], Xe[edges]) @ W2 + b2, vertex, N)
  out = ((1-a)*Xv + a*Xd) @ W + b

Algebraic reassociation (exact up to fp reassociation), with
U1=(1-a)W2top@W, U2=(1-a)W2bot@W, U3=aW, V1=W1@U2, c1=b1@U2,
c2=(1-a)(b2@W):
  A[e]  = segment_sum(Xd[vertex], edges, E)
  Z     = A @ V1 + cnt_e x c1          (= Xe @ U2)
  B'[v] = segment_sum(Z[edges], vertex, N)
  out   = cnt_v o (Xd@U1) + Xd@U3 + B' + cnt_v x c2 + b

Distribution: nodes sharded 8 ways by row range; incidence lists bucketed by
destination core; the only collective is an AllReduce of the per-core partial
A^T [EG*64,128] bf16. Segment sums run as one-hot matmul accumulation in PSUM
over 128-destination groups; gathers use dma_gather (int16 idx, <=1024/call).
Step 2 uses swapped matmul operands (rows stationary, one-hot moving) so A is
produced transposed and step 3 needs no transposes. Step 5 is fused into
step 4's PSUM accumulation groups.
"""
import sys
import numpy as np

sys.path.insert(0, "/opt/trn_rl_repo")

D = 64
NC = 8
CHUNK = 1024        # dma_gather per-call token cap
MB = 16             # one-hot tiles built per DVE op
ALPHA = 0.5
BUCKET = 32768      # int16 gather index range
TRACE = False
LAST_EXEC_NS = None
LAST_RESULTS = None


def _wrap16(a):
    a = np.asarray(a, np.int16)
    return np.tile(a.reshape(-1, 16).T, (8, 1))  # [128, T/16]


def _wrap128(a):
    return np.ascontiguousarray(np.asarray(a).reshape(-1, 128).T)  # [128, T/128]


def _prep(inputs, n_edges):
    X = np.ascontiguousarray(np.asarray(inputs["X"], np.float32))
    N = X.shape[0]
    assert N % NC == 0
    S = N // NC
    G1 = -(-S // 128)          # node groups per core
    SP = G1 * 128
    EG = -(-n_edges // 128)    # edge groups
    EP = EG * 128
    NB = -(-N // BUCKET)

    dr = np.asarray(inputs["dif_rows"], np.int64)
    dc = np.asarray(inputs["dif_cols"], np.int64)
    dv = np.asarray(inputs["dif_vals"], np.float32)
    vx = np.asarray(inputs["vertex"], np.int64)
    eg = np.asarray(inputs["edges"], np.int64)
    assert eg.max() < n_edges and vx.max() < N and dr.max() < N and dc.max() < N

    # --- per-cell tile plans (max fill over cores; uniform across cores) ---
    def plan(core, cell, ncells, min_one):
        cnt = np.bincount(core * ncells + cell,
                          minlength=NC * ncells).reshape(NC, ncells)
        k = -(-cnt.max(0) // 128)
        k = np.maximum(k, min_one)
        off = np.zeros(ncells + 1, np.int64)
        np.cumsum(k, out=off[1:])
        return k, off * 128, int(off[-1]) * 128

    c1 = dr // S
    min1 = np.zeros(NB * G1, np.int64)
    min1[:G1] = 1  # bucket-0 cells init the Xd accumulator
    kc1, off1, T1 = plan(c1, (dc // BUCKET) * G1 + (dr % S) // 128,
                         NB * G1, min1)
    c2 = vx // S
    kc2, off2, T2 = plan(c2, eg // 128, EG, 1)
    kc4, off4, T4 = plan(c2, (vx % S) // 128, G1, 1)
    T1 = -(-T1 // 2048) * 2048  # keep /16 and /128 wrappings integral
    T2 = -(-T2 // 2048) * 2048
    T4 = -(-T4 // 2048) * 2048

    import ml_dtypes
    bf = ml_dtypes.bfloat16
    Wf = np.asarray(inputs["W_w"], np.float32)
    W1 = np.asarray(inputs["W1_w"], np.float32)
    W2 = np.asarray(inputs["W2_w"], np.float32)
    b1 = np.asarray(inputs["W1_b"], np.float32)
    b2 = np.asarray(inputs["W2_b"], np.float32)
    bb = np.asarray(inputs["W_b"], np.float32)
    U1 = (1.0 - ALPHA) * (W2[:D] @ Wf)
    U2 = (1.0 - ALPHA) * (W2[D:] @ Wf)
    U3 = ALPHA * Wf
    V1 = W1 @ U2
    c1row = b1 @ U2
    c2row = (1.0 - ALPHA) * (b2 @ Wf)

    shared = {
        "X": X,
        "U1": np.ascontiguousarray(U1).astype(bf),
        "U3": np.ascontiguousarray(U3).astype(bf),
        "V1": np.ascontiguousarray(V1).astype(bf),
        "c1_rep": np.tile(c1row, (128, 1)).astype(np.float32),
        "c2b": np.ascontiguousarray(np.stack([c2row, bb])).astype(bf),
        "cnte": _wrap128(np.bincount(eg, minlength=EP).astype(np.float32)),
        "iota16": np.ascontiguousarray(
            np.tile(np.arange(128, dtype=bf), (128, MB))),  # [128, MB*128]
        "ident": np.eye(128).astype(bf),
    }

    def fill(slots_T, cell_of_tok, kcell, offs, order, gval, dval, vval=None):
        # slots_T: total slots; cell size kcell*128; tokens sorted by `order`.
        cell = cell_of_tok[order]
        g = gval[order]
        d = dval[order]
        if len(cell):
            newc = np.empty(len(cell), bool)
            newc[0] = True
            newc[1:] = cell[1:] != cell[:-1]
            starts = np.where(newc)[0]
            idx = np.arange(len(cell))
            cell_start = np.zeros(len(cell), np.int64)
            cell_start[starts] = idx[starts]
            cell_start = np.maximum.accumulate(cell_start)
            rank = idx - cell_start
        else:
            rank = np.zeros(0, np.int64)
        slot = offs[cell] + rank
        assert len(slot) == 0 or (rank < kcell[cell] * 128).all()
        gi = np.zeros(slots_T, np.int64)
        dl = np.full(slots_T, -1.0, np.float32)
        gi[slot] = g
        dl[slot] = d
        import ml_dtypes as _md
        out = [_wrap16(gi), _wrap128(dl.astype(_md.bfloat16))]
        if vval is not None:
            vv = np.zeros(slots_T, np.float32)
            vv[slot] = vval[order]
            out.append(_wrap128(vv))
        return out

    in_maps = []
    for c in range(NC):
        lo = c * S
        m = (dr >= lo) & (dr < lo + S)
        d1 = dr[m] - lo
        c1_, v1 = dc[m], dv[m]
        b1_ = c1_ // BUCKET
        cell1 = b1_ * G1 + d1 // 128  # bucket-major cell id
        order1 = np.lexsort((c1_, cell1))   # within cell: by source column
        gi1, dl1, vv1 = fill(T1, cell1, kc1, off1, order1, c1_ - b1_ * BUCKET,
                             d1 % 128, v1)

        m2 = (vx >= lo) & (vx < lo + S)
        e2, v2 = eg[m2], vx[m2] - lo
        order2 = np.lexsort((v2, e2 // 128))  # within eg-cell: by source v
        gi2, dl2 = fill(T2, e2 // 128, kc2, off2, order2, v2, e2 % 128)
        order4 = np.lexsort((e2, v2 // 128))  # within vg-cell: by source e
        gi4, dl4 = fill(T4, v2 // 128, kc4, off4, order4, e2, v2 % 128)

        cntv = np.bincount(v2, minlength=SP).astype(np.float32)
        cnt_ones = np.ascontiguousarray(
            np.stack([cntv, np.ones(SP, np.float32)])).astype(bf)
        in_maps.append(dict(shared,
                            gidx1=gi1, drel1=dl1, val1=vv1,
                            gidx2=gi2, drel2=dl2,
                            gidx4=gi4, drel4=dl4,
                            cntv=_wrap128(cntv), cnt_ones=cnt_ones))

    meta = dict(N=N, S=S, G1=G1, SP=SP, EG=EG, EP=EP, NB=NB,
                KC1=kc1.tolist(), OFF1=off1.tolist(),
                KC2=kc2.tolist(), OFF2=off2.tolist(),
                KC4=kc4.tolist(), OFF4=off4.tolist(),
                T1=T1, T2=T2, T4=T4)
    return meta, in_maps


def _build(meta):
    from concourse import bass, bacc, tile, mybir

    f32, i16 = mybir.dt.float32, mybir.dt.int16
    bf16 = mybir.dt.bfloat16
    N, S, G1, SP, EG, EP, NB = (meta[k] for k in
                                ("N", "S", "G1", "SP", "EG", "EP", "NB"))
    T1, T2, T4 = meta["T1"], meta["T2"], meta["T4"]
    KC1, OFF1 = meta["KC1"], meta["OFF1"]
    KC2, OFF2 = meta["KC2"], meta["OFF2"]
    KC4, OFF4 = meta["KC4"], meta["OFF4"]

    nc = bacc.Bacc("TRN2", target_bir_lowering=False, debug=False,
                   num_devices=NC, num_swdge_queues=4)

    def par(name, shape, dt=f32, out=False):
        return nc.declare_dram_parameter(name, list(shape), dt, isOutput=out)

    Xp = par("X", (N, D))
    gidx1 = par("gidx1", (128, T1 // 16), i16)
    drel1 = par("drel1", (128, T1 // 128), bf16)
    val1 = par("val1", (128, T1 // 128))
    gidx2 = par("gidx2", (128, T2 // 16), i16)
    drel2 = par("drel2", (128, T2 // 128), bf16)
    gidx4 = par("gidx4", (128, T4 // 16), i16)
    drel4 = par("drel4", (128, T4 // 128), bf16)
    cntv = par("cntv", (128, G1))
    cnt_ones = par("cnt_ones", (2, SP), bf16)
    cnte = par("cnte", (128, EG))
    U1p = par("U1", (D, D), bf16)
    U3p = par("U3", (D, D), bf16)
    V1p = par("V1", (D, D), bf16)
    c1_rep = par("c1_rep", (128, D))
    c2bp = par("c2b", (2, D), bf16)
    iota16 = par("iota16", (128, MB * 128), bf16)
    ident = par("ident", (128, 128), bf16)
    OUT = par("OUT", (SP, D), out=True)

    eq = mybir.AluOpType.is_equal
    mult = mybir.AluOpType.mult
    addop = mybir.AluOpType.add
    CopyF = mybir.ActivationFunctionType.Copy

    with tile.TileContext(nc) as tc:
        with (
            tc.tile_pool(name="meta1", bufs=1) as metap,
            tc.tile_pool(name="gpool", bufs=12) as gpool,
            tc.tile_pool(name="mpool", bufs=4) as mpool,
            tc.tile_pool(name="psA", bufs=3, space="PSUM") as psA,
            tc.tile_pool(name="psB", bufs=2, space="PSUM") as psB,
            tc.tile_pool(name="psT", bufs=2, space="PSUM") as psT,
            tc.tile_pool(name="stage", bufs=3) as stage,
            tc.tile_pool(name="dram", bufs=1, space="DRAM") as dram,
        ):
            # --- resident metadata ---
            def load(ap_param, shape, nm, dt=f32, pool=metap):
                t = pool.tile(list(shape), dt, name=nm, tag=nm)
                nc.scalar.dma_start(t[:], ap_param[:])
                return t

            iota_t = load(iota16, (128, MB, 128), "iota_t", dt=bf16)
            ident_t = load(ident, (128, 128), "ident_t", dt=bf16)
            u1_t = load(U1p, (D, D), "u1_t", dt=bf16)
            u3_t = load(U3p, (D, D), "u3_t", dt=bf16)
            v1_t = load(V1p, (D, D), "v1_t", dt=bf16)
            c1_t = load(c1_rep, (128, D), "c1_t")
            c2b_t = load(c2bp, (2, D), "c2b_t", dt=bf16)
            cntv_t = load(cntv, (128, G1), "cntv_t")
            cnto_t = load(cnt_ones, (2, SP), "cnto_t", dt=bf16)
            cnte_t = load(cnte, (128, EG), "cnte_t")
            drel1_t = load(drel1, (128, T1 // 128), "drel1_t", dt=bf16)
            val1_t = load(val1, (128, T1 // 128), "val1_t")
            drel2_t = load(drel2, (128, T2 // 128), "drel2_t", dt=bf16)
            drel4_t = load(drel4, (128, T4 // 128), "drel4_t", dt=bf16)
            gidx1_t = metap.tile([128, T1 // 16], i16, name="gidx1_t",
                                 tag="gidx1_t")
            nc.sync.dma_start(gidx1_t[:], gidx1[:])
            gidx2_t = metap.tile([128, T2 // 16], i16, name="gidx2_t",
                                 tag="gidx2_t")
            nc.sync.dma_start(gidx2_t[:], gidx2[:])
            gidx4_t = metap.tile([128, T4 // 16], i16, name="gidx4_t",
                                 tag="gidx4_t")
            nc.sync.dma_start(gidx4_t[:], gidx4[:])

            Xd_sb = metap.tile([128, G1, D], f32)     # wrapped node shard
            XdB_sb = metap.tile([128, G1, D], bf16)
            xdT_all = metap.tile([D, G1, 128], bf16)  # per-group Xd^T
            Xd_hbm = dram.tile([SP, 2 * D], bf16)
            Z_hbm = dram.tile([EP, 2 * D], bf16)
            EGH = (EG // 2) // 4 * 4  # first-half edge groups (AllReduce split)
            A_part1 = dram.tile([EGH * D, 128], bf16)
            A_part2 = dram.tile([(EG - EGH) * D, 128], bf16)
            A_full1 = dram.tile([EGH * D, 128], bf16, addr_space="Shared")
            A_full2 = dram.tile([(EG - EGH) * D, 128], bf16,
                                addr_space="Shared")
            qctr = [0]

            def sparse_step(gidx_t, gidx_cols, drel_t, val_t, srcs,
                            kcells, offs, evac, src_dt, src_cols, swap,
                            open_group=False, cell_lo=0, cell_hi=None):
                """srcs: (src_ap, cell_lo, cell_hi) bucket streams covering
                cells [lo, hi); slot spans from offs. swap: rows stationary,
                one-hot moving -> psum [64,128]. open_group: leave the PSUM
                matmul group open (evac closes it)."""
                if cell_hi is None:
                    cell_hi = len(kcells)
                tile_src = {}
                for src_ap, c_lo, c_hi in srcs:
                    base, end = offs[c_lo], offs[c_hi]
                    off = 0
                    L = end - base
                    while off < L:
                        n = min(CHUNK, L - off)
                        cols = n // 128
                        gt = gpool.tile([128, CHUNK // 128, src_cols], src_dt,
                                        tag="g")
                        nc.gpsimd.dma_gather(
                            gt[:, :cols, :], src_ap,
                            gidx_t[:, (base + off) // 16:(base + off + n) // 16],
                            n, n, src_cols, queue_num=qctr[0] % 4)
                        qctr[0] += 1
                        if val_t is not None:
                            g2 = gpool.tile([128, CHUNK // 128, D], bf16,
                                            tag="g2")
                            vs = val_t[:, (base + off) // 128:
                                       (base + off) // 128 + cols]
                            nc.vector.tensor_mul(
                                g2[:, :cols, :], gt[:, :cols, :D],
                                vs.unsqueeze(2).broadcast_to([128, cols, D]))
                            src_t = g2
                        else:
                            src_t = gt
                        for i in range(cols):
                            tile_src[(base + off) // 128 + i] = (src_t, i)
                        off += n
                ntiles = offs[len(kcells)] // 128
                mb_next = 0
                m_buf = None
                mb_base = 0
                for cell in range(cell_lo, cell_hi):
                    kt = kcells[cell]
                    if kt == 0:
                        continue
                    cur_full = psA.tile([128, 128], f32, tag="acc")
                    cur = cur_full[0:D, :] if swap else cur_full[:, 0:D]
                    t0 = offs[cell] // 128
                    for i in range(kt):
                        t = t0 + i
                        if t >= mb_next:
                            k = min(MB, ntiles - t)
                            m_buf = mpool.tile([128, MB, 128],
                                              mybir.dt.float8e4, tag="m")
                            db = drel_t[:, t:t + k].unsqueeze(2).broadcast_to(
                                [128, k, 128])
                            nc.vector.tensor_tensor(m_buf[:, :k, :],
                                                    iota_t[:, :k, :], db, eq)
                            mb_base, mb_next = t, t + k
                        gt, col = tile_src[t]
                        last = (i == kt - 1) and not open_group
                        if swap:
                            nc.tensor.matmul(cur[:], gt[:, col, :D],
                                             m_buf[:, t - mb_base, :],
                                             start=(i == 0), stop=last)
                        else:
                            nc.tensor.matmul(cur[:], m_buf[:, t - mb_base, :],
                                             gt[:, col, :D],
                                             start=(i == 0), stop=last)
                    evac(cell, cur)

            # ---- step 1: diffusion into Xd ----
            srcs1 = []
            for b in range(NB):
                rows = min(BUCKET, N - b * BUCKET)
                srcs1.append((Xp[b * BUCKET:b * BUCKET + rows, :],
                              b * G1, (b + 1) * G1))

            def evac1(cellidx, psum):
                b, g = divmod(cellidx, G1)
                if b == 0:
                    nc.scalar.activation(Xd_sb[:, g, :], psum[:], CopyF)
                else:
                    nc.vector.tensor_add(Xd_sb[:, g, :], Xd_sb[:, g, :],
                                         psum[:])
                if b == NB - 1:
                    nc.scalar.activation(XdB_sb[:, g, :], Xd_sb[:, g, :],
                                         CopyF)

            sparse_step(gidx1_t, T1 // 16, drel1_t, val1_t, srcs1, KC1, OFF1,
                        evac1, f32, D, swap=False)

            # Xd wrapped -> row-major bf16 HBM table (step-2 gather source);
            # chunked so early groups upload while late cells still compute
            WCH = 14
            for g0 in range(0, G1, WCH):
                gn = min(WCH, G1 - g0)
                nc.sync.dma_start(
                    Xd_hbm[g0 * 128:(g0 + gn) * 128, :D]
                    .rearrange("(g p) f -> p g f", p=128),
                    XdB_sb[:, g0:g0 + gn, :])

            # ---- step 2: A^T[e] partials (rows stationary, one-hot moving) --
            ev2 = {}

            def evac2(g, psum):
                b = g % 4
                if b == 0:
                    ev2["t"] = stage.tile([D, 4, 128], bf16, tag="ev2",
                                          name="ev2t")
                    ev2["g0"] = g
                nc.scalar.activation(ev2["t"][:, b, :], psum[:], CopyF)
                if b == 3 or g == EG - 1:
                    nb = b + 1
                    g0 = ev2["g0"]
                    dst, base = ((A_part1, 0) if g0 < EGH
                                 else (A_part2, EGH))
                    nc.sync.dma_start(
                        dst[(g0 - base) * D:(g0 - base + nb) * D, :]
                        .rearrange("(b p) f -> p b f", p=D),
                        ev2["t"][:, :nb, :])

            sparse_step(gidx2_t, T2 // 16, drel2_t, None,
                        [(Xd_hbm[:, :], 0, EGH)], KC2, OFF2, evac2,
                        bf16, 2 * D, swap=True, cell_lo=0, cell_hi=EGH)
            # AR of the first half runs on the CC cores while the second
            # half's gathers stream
            nc.gpsimd.collective_compute(
                "AllReduce", addop,
                replica_groups=[list(range(NC))],
                ins=[A_part1.opt()], outs=[A_full1.opt()])
            sparse_step(gidx2_t, T2 // 16, drel2_t, None,
                        [(Xd_hbm[:, :], EGH, EG)], KC2, OFF2, evac2,
                        bf16, 2 * D, swap=True, cell_lo=EGH, cell_hi=EG)

            # ---- AllReduce A^T (second half) ----
            nc.gpsimd.collective_compute(
                "AllReduce", addop,
                replica_groups=[list(range(NC))],
                ins=[A_part2.opt()], outs=[A_full2.opt()])

            # per-group Xd^T for step 5 (PE transpose, Act evac) -- issued
            # here so it overlaps step-2 gathers / the collectives
            for g in range(G1):
                pT = psT.tile([D, 128], bf16, tag="t")
                nc.tensor.transpose(pT[:], XdB_sb[:, g, :], ident_t[:])
                nc.scalar.activation(xdT_all[:, g, :], pT[:], CopyF)

            # ---- step 3: Z = A @ V1 + cnt_e x c1 (no transposes) ----
            ev3 = {}
            a4 = None
            for g in range(EG):
                if g % 4 == 0:
                    nb4 = min(4, EG - g)
                    a4 = stage.tile([D, 4, 128], bf16, tag="a", name="a4")
                    src_t, sbase = ((A_full1, 0) if g < EGH
                                    else (A_full2, EGH))
                    nc.sync.dma_start(
                        a4[:, :nb4, :],
                        src_t[(g - sbase) * D:(g - sbase + nb4) * D, :]
                        .rearrange("(b p) f -> p b f", p=D))
                p2 = psB.tile([128, D], f32, tag="acc1")
                nc.tensor.matmul(p2[:], a4[:, g % 4, :], v1_t[:],
                                 start=True, stop=True)
                b4 = g % 4
                if b4 == 0:
                    ev3["t"] = stage.tile([128, 4, D], bf16, tag="ev3",
                                          name="ev3t")
                    ev3["g0"] = g
                nc.vector.scalar_tensor_tensor(
                    ev3["t"][:, b4, :], c1_t[:], cnte_t[:, g:g + 1], p2[:],
                    mult, addop)
                if b4 == 3 or g == EG - 1:
                    nb = b4 + 1
                    nc.sync.dma_start(
                        Z_hbm[ev3["g0"] * 128:(ev3["g0"] + nb) * 128, :D]
                        .rearrange("(b p) f -> p b f", p=128),
                        ev3["t"][:, :nb, :])

            # ---- step 4 + step 5 fused ----
            otb = [None]

            def evac4(g, psum):
                # append:  + Xd[g]@U3  + cnt_v x c2 + 1 x b   (then stop)
                nc.tensor.matmul(psum[:], xdT_all[:, g, :], u3_t[:],
                                 start=False, stop=False)
                nc.tensor.matmul(psum[:], cnto_t[:, g * 128:(g + 1) * 128],
                                 c2b_t[:], start=False, stop=True)
                p1 = psB.tile([128, D], f32, tag="acc1")
                nc.tensor.matmul(p1[:], xdT_all[:, g, :], u1_t[:],
                                 start=True, stop=True)
                t1 = stage.tile([128, D], f32, tag="t1")
                nc.scalar.activation(t1[:], p1[:], CopyF)
                b4 = g % 4
                if b4 == 0:
                    otb[0] = stage.tile([128, 4, D], f32, tag="otb",
                                        name="otb")
                nc.vector.scalar_tensor_tensor(
                    otb[0][:, b4, :], t1[:], cntv_t[:, g:g + 1], psum[:],
                    mult, addop)
                if b4 == 3 or g == G1 - 1:
                    nb = b4 + 1
                    nc.sync.dma_start(
                        OUT[(g - nb + 1) * 128:(g + 1) * 128, :]
                        .rearrange("(b p) f -> p b f", p=128),
                        otb[0][:, :nb, :])

            sparse_step(gidx4_t, T4 // 16, drel4_t, None,
                        [(Z_hbm[:, :], 0, G1)], KC4, OFF4, evac4,
                        bf16, 2 * D, swap=False, open_group=True)

    nc.compile()
    return nc


def _run(inputs, n_edges, sim=False):
    meta, in_maps = _prep(inputs, n_edges)
    nc = _build(meta)
    S, SP = meta["S"], meta["SP"]
    if sim:
        from concourse import bass_interp
        ms = bass_interp.MultiCoreSim(nc, NC, require_finite=False,
                                      require_nnan=False)
        for c in range(NC):
            for k, v in in_maps[c].items():
                ms.cores[c].tensor(k)[:] = v
        ms.simulate()
        outs = [np.array(ms.cores[c].mem_tensor("OUT")).reshape(SP, D)
                for c in range(NC)]
    else:
        from concourse.bass_utils import run_bass_kernel_spmd
        res = run_bass_kernel_spmd(nc, in_maps, list(range(NC)),
                                   trace=TRACE)
        global LAST_EXEC_NS, LAST_RESULTS
        LAST_EXEC_NS = res.exec_time_ns
        LAST_RESULTS = res
        outs = [res.results[c]["OUT"] for c in range(NC)]
    return np.concatenate([o[:S] for o in outs], axis=0).astype(np.float32)


def kernel(**inputs):
    return _run(inputs, 25000, sim=False)
